# revision 10
# baseline (speedup 1.0000x reference)
"""Trainium2 Bass kernel: DistributedPiKVMoE (top-2 MoE block, 8 experts).

Expert-parallel with sparse dispatch (8 cores = 8 experts):
  - The host runs a shadow fp32 router only to build per-(expert, batch)
    token index lists (capacity 384 per batch, padded with not-assigned
    tokens) and gathers/transposes the selected tokens' activations.
  - Each core computes, for its expert: the replicated fp32 router
    (exact top-2 gates + load-balance stats), k/v projections over ALL
    tokens (attention keys), and the q/FFN/attention/o pipeline only
    for its selected tokens.  Gates are gathered back from the device
    router output via indirect DMA, so the masked combine uses the
    device's fp32 selection, and padded slots get an exact 0 gate.
  - The host scatter-adds the 8 cores' gated partial outputs (the
    all-reduce of the reference module) and reduces stats -> lb_loss.
  - Expert matmuls run in bf16 with fp32 PSUM accumulation; layouts
    avoid any on-device transpose (x arrives pre-transposed; scores are
    computed transposed [k, q]; softmax via exp + ones-matmul
    denominator; 1/denom and gate fold into the o-projection eviction).

Shapes hardcoded for B=4, S=1024, H=512, E=8, F=2048, TOPK=2.
"""

import numpy as np
import ml_dtypes

B, S, H, E, F = 4, 1024, 512, 8, 2048
T = B * S            # 4096 tokens
HC = H // 128        # 4 h-chunks of 128
FC = F // 128        # 16 f-chunks of 128
G = T // 512         # 8 router token groups of 512
GB = S // 512        # 2 groups per batch
CB = 384             # per-(expert, batch) token capacity (mean ~256)
ST = CB // 128       # 3 subtiles of selected tokens per batch
NEG = -1.0e30

_NC_CACHE = {}


def _build_nc():
    import concourse.tile as tile
    from concourse import bacc, mybir
    from concourse.bass import ts, IndirectOffsetOnAxis

    f32 = mybir.dt.float32
    bf16 = mybir.dt.bfloat16
    i32 = mybir.dt.int32
    AF = mybir.ActivationFunctionType
    OP = mybir.AluOpType
    AX = mybir.AxisListType

    nc = bacc.Bacc("TRN2", target_bir_lowering=False, debug=False)

    # ---- DRAM I/O ----
    xT32 = nc.dram_tensor("xT32", [HC, 128, T], f32, kind="ExternalInput")
    xT16 = nc.dram_tensor("xT16", [HC, 128, T], bf16, kind="ExternalInput")
    xsel_d = nc.dram_tensor("xsel", [HC, 128, B * CB], bf16, kind="ExternalInput")
    xsel32_d = nc.dram_tensor("xsel32", [HC, 128, B * CB], f32, kind="ExternalInput")
    wg_d = nc.dram_tensor("wg", [HC, 128, E], f32, kind="ExternalInput")
    bgb_d = nc.dram_tensor("bgb", [128, 4, E], f32, kind="ExternalInput")
    esel_d = nc.dram_tensor("esel", [128, 4, E], f32, kind="ExternalInput")
    w1_d = nc.dram_tensor("w1", [HC, 128, F], bf16, kind="ExternalInput")
    b1_d = nc.dram_tensor("b1", [128, FC], f32, kind="ExternalInput")
    w2_d = nc.dram_tensor("w2", [FC, 128, H], bf16, kind="ExternalInput")
    wq_d = nc.dram_tensor("wq", [HC, 128, H], bf16, kind="ExternalInput")
    wk_d = nc.dram_tensor("wk", [HC, 128, H], bf16, kind="ExternalInput")
    wv_d = nc.dram_tensor("wv", [HC, 128, H], bf16, kind="ExternalInput")
    wo_d = nc.dram_tensor("wo", [HC, 128, H], bf16, kind="ExternalInput")
    qkb_d = nc.dram_tensor("qkb", [128, 2 * HC], f32, kind="ExternalInput")
    vbb_d = nc.dram_tensor("vbb", [128, H], f32, kind="ExternalInput")
    b2ob_d = nc.dram_tensor("b2ob", [128, H], f32, kind="ExternalInput")
    SCALE = float(np.float32(1.0) / np.sqrt(np.float32(H)))

    out_d = nc.dram_tensor("out", [B * CB, H], f32, kind="ExternalOutput")
    stats_d = nc.dram_tensor("stats", [128, 8], f32, kind="ExternalOutput")
    rscr_d = nc.dram_tensor("rscr", [B, CB], f32, kind="ExternalOutput")

    with tile.TileContext(nc) as tc:
        with (
            tc.tile_pool(name="const", bufs=1) as cp,
            tc.tile_pool(name="xr", bufs=2) as xrp,
            tc.tile_pool(name="xg", bufs=2) as xgp,
            tc.tile_pool(name="xs", bufs=2) as xsp,
            tc.tile_pool(name="h1", bufs=1) as h1p,
            tc.tile_pool(name="ffn", bufs=2) as ffnp,
            tc.tile_pool(name="qq", bufs=2) as qp,
            tc.tile_pool(name="kk", bufs=1) as kp,
            tc.tile_pool(name="vv", bufs=1) as vp,
            tc.tile_pool(name="es", bufs=2) as esp,
            tc.tile_pool(name="at", bufs=2) as atp,
            tc.tile_pool(name="rr", bufs=2) as rp,
            tc.tile_pool(name="gg", bufs=8) as gp,
            tc.tile_pool(name="x3", bufs=2) as x3p,
            tc.tile_pool(name="ou", bufs=4) as outp,
            tc.tile_pool(name="rt", bufs=2) as rtp,
            tc.tile_pool(name="psB", bufs=4, space="PSUM") as psB,
            tc.tile_pool(name="psS", bufs=2, space="PSUM") as psS,
        ):
            # ---- tiny constants ----
            wg_sb = cp.tile([128, HC, E], f32, tag="wg")
            for c in range(HC):
                nc.sync.dma_start(wg_sb[:, c, :], wg_d[c])
            bgb = cp.tile([128, 4, E], f32, tag="bgb")
            nc.sync.dma_start(bgb[:], bgb_d[:])
            esel = cp.tile([128, 4, E], f32, tag="esel")
            nc.sync.dma_start(esel[:], esel_d[:])
            b1_sb = cp.tile([128, FC], f32, tag="b1")
            nc.sync.dma_start(b1_sb[:], b1_d[:])
            qkb = cp.tile([128, 2 * HC], f32, tag="qkb")
            nc.sync.dma_start(qkb[:], qkb_d[:])
            vbb = cp.tile([128, H], f32, tag="vbb")
            nc.sync.dma_start(vbb[:], vbb_d[:])
            b2ob = cp.tile([128, H], f32, tag="b2ob")
            nc.sync.dma_start(b2ob[:], b2ob_d[:])

            ones_bf = cp.tile([128, 1], bf16, tag="ones")
            nc.vector.memset(ones_bf[:], 1.0)
            accm = cp.tile([128, 4], f32, tag="accm")
            accp = cp.tile([128, 4], f32, tag="accp")
            nc.vector.memset(accm[:], 0.0)
            nc.vector.memset(accp[:], 0.0)

            # ---- big weights (emission position controls DMA priority) ----
            wk_sb = cp.tile([128, HC, H], bf16, tag="wk")
            wv_sb = cp.tile([128, HC, H], bf16, tag="wv")
            for c in range(HC):
                nc.sync.dma_start(wk_sb[:, c, :], wk_d[c])
                nc.sync.dma_start(wv_sb[:, c, :], wv_d[c])
            w1_sb = cp.tile([128, HC, F], bf16, tag="w1")
            for c in range(HC):
                nc.sync.dma_start(w1_sb[:, c, :], w1_d[c])

            # ---------------- emission helpers ----------------
            kT = {}
            vsb = {}
            xsel = {}
            h1t = {}
            fft = {}
            qTs = {}
            est = {}
            ats = {}
            rbt = {}

            def emit_kv(b, gl):
                g = b * GB + gl
                xg = xgp.tile([128, HC, 512], bf16, tag="xg")
                for c in range(HC):
                    nc.sync.dma_start(xg[:, c, :], xT16[c][:, ts(g, 512)])
                if gl == 0:
                    kT[b] = kp.tile([128, HC, S], bf16, tag="kT", name="kTt")
                    vsb[b] = vp.tile([128, 8, H], bf16, tag="v", name="vt")
                for dc in range(HC):
                    ps = psB.tile([128, 512], f32, tag="ps")
                    for c in range(HC):
                        nc.tensor.matmul(
                            ps[:],
                            wk_sb[:, c, ts(dc, 128)],
                            xg[:, c, :],
                            start=(c == 0),
                            stop=(c == HC - 1),
                        )
                    nc.vector.tensor_scalar_add(
                        kT[b][:, dc, ts(gl, 512)], ps[:], qkb[:, HC + dc : HC + dc + 1]
                    )
                for t in range(4):
                    ps = psB.tile([128, 512], f32, tag="ps")
                    for c in range(HC):
                        nc.tensor.matmul(
                            ps[:],
                            xg[:, c, ts(t, 128)],
                            wv_sb[:, c, :],
                            start=(c == 0),
                            stop=(c == HC - 1),
                        )
                    nc.vector.tensor_tensor(
                        vsb[b][:, gl * 4 + t, :], ps[:], vbb[:], op=OP.add
                    )

            def emit_fc1(b):
                xs = xsp.tile([128, HC, CB], bf16, tag="xs")
                xsel[b] = xs
                for c in range(HC):
                    nc.sync.dma_start(xs[:, c, :], xsel_d[c][:, ts(b, CB)])
                h1 = h1p.tile([128, FC, CB], bf16, tag="h1")
                h1t[b] = h1
                for fc in range(FC):
                    ps = psB.tile([128, 512], f32, tag="ps")
                    for c in range(HC):
                        nc.tensor.matmul(
                            ps[:, :CB],
                            w1_sb[:, c, ts(fc, 128)],
                            xs[:, c, :],
                            start=(c == 0),
                            stop=(c == HC - 1),
                        )
                    nc.scalar.activation(
                        h1[:, fc, :], ps[:, :CB], AF.Gelu, bias=b1_sb[:, fc : fc + 1]
                    )

            def emit_fc2(b):
                h1 = h1t[b]
                fsb = ffnp.tile([128, ST, H], f32, tag="ffn")
                fft[b] = fsb
                for t in range(ST):
                    ps = psB.tile([128, 512], f32, tag="ps")
                    for fc in range(FC):
                        nc.tensor.matmul(
                            ps[:],
                            h1[:, fc, ts(t, 128)],
                            w2_sb[:, fc, :],
                            start=(fc == 0),
                            stop=(fc == FC - 1),
                        )
                    nc.vector.tensor_tensor(fsb[:, t, :], ps[:], b2ob[:], op=OP.add)

            def emit_q(b):
                xs = xsel[b]
                qT = qp.tile([128, HC, CB], bf16, tag="qT")
                qTs[b] = qT
                for dc in range(HC):
                    ps = psB.tile([128, 512], f32, tag="ps")
                    for c in range(HC):
                        nc.tensor.matmul(
                            ps[:, :CB],
                            wq_sb[:, c, ts(dc, 128)],
                            xs[:, c, :],
                            start=(c == 0),
                            stop=(c == HC - 1),
                        )
                    nc.vector.tensor_scalar(
                        qT[:, dc, :],
                        ps[:, :CB],
                        qkb[:, dc : dc + 1],
                        SCALE,
                        op0=OP.add,
                        op1=OP.mult,
                    )

            def emit_attn(b):
                es = esp.tile([128, 8, CB], bf16, tag="es")
                est[b] = es
                for kc in range(8):
                    ps = psB.tile([128, 512], f32, tag="ps")
                    for dc in range(HC):
                        nc.tensor.matmul(
                            ps[:, :CB],
                            kT[b][:, dc, ts(kc, 128)],
                            qTs[b][:, dc, :],
                            start=(dc == 0),
                            stop=(dc == HC - 1),
                        )
                    nc.scalar.activation(es[:, kc, :], ps[:, :CB], AF.Exp)
                # softmax denominators for the selected queries
                rrow = rp.tile([1, CB], f32, tag="rrow")
                psd = psS.tile([1, 512], f32, tag="psS")
                for kc in range(8):
                    nc.tensor.matmul(
                        psd[:, :CB],
                        ones_bf[:],
                        es[:, kc, :],
                        start=(kc == 0),
                        stop=(kc == 7),
                    )
                nc.vector.reciprocal(rrow[:], psd[:, :CB])
                nc.sync.dma_start(rscr_d[b : b + 1, :], rrow[:])
                rb = rp.tile([128, ST], f32, tag="rb")
                rbt[b] = rb
                nc.sync.dma_start(
                    rb[:], rscr_d[b : b + 1, :].rearrange("a (c p) -> (a p) c", p=128)
                )
                at = atp.tile([128, HC, CB], bf16, tag="at")
                ats[b] = at
                for dc in range(HC):
                    ps = psB.tile([128, 512], f32, tag="ps")
                    for kc in range(8):
                        nc.tensor.matmul(
                            ps[:, :CB],
                            vsb[b][:, kc, ts(dc, 128)],
                            es[:, kc, :],
                            start=(kc == 0),
                            stop=(kc == 7),
                        )
                    nc.vector.tensor_copy(at[:, dc, :], ps[:, :CB])

            gst = {}

            def emit_selrouter(b):
                xs32 = x3p.tile([128, HC, CB], f32, tag="xs32", name="xs32t")
                for c in range(HC):
                    nc.sync.dma_start(xs32[:, c, :], xsel32_d[c][:, ts(b, CB)])
                lg = psS.tile([128, ST, E], f32, tag="psS", name="lgs")
                for t in range(ST):
                    for c in range(HC):
                        nc.tensor.matmul(
                            lg[:, t, :],
                            xs32[:, c, ts(t, 128)],
                            wg_sb[:, c, :],
                            start=(c == 0),
                            stop=(c == HC - 1),
                        )
                logit = rtp.tile([128, ST, E], f32, tag="slg", name="slg")
                nc.vector.tensor_tensor(logit[:], lg[:], bgb[:, 0:ST, :], op=OP.add)
                expl = rtp.tile([128, ST, E], f32, tag="sex", name="sex")
                nc.scalar.activation(expl[:], logit[:], AF.Exp)
                m1 = rtp.tile([128, ST, 1], f32, tag="sm1", name="sm1")
                nc.vector.reduce_max(m1[:], logit[:], axis=AX.X)
                mask1 = rtp.tile([128, ST, E], f32, tag="smk", name="smk")
                nc.vector.tensor_tensor(
                    mask1[:], logit[:], m1[:].broadcast_to((128, ST, E)), op=OP.is_ge
                )
                pmask = rtp.tile([128, ST, E], f32, tag="spm", name="spm")
                nc.vector.scalar_tensor_tensor(
                    pmask[:], mask1[:], NEG, logit[:], op0=OP.mult, op1=OP.add
                )
                m2 = rtp.tile([128, ST, 1], f32, tag="sm2", name="sm2")
                nc.vector.reduce_max(m2[:], pmask[:], axis=AX.X)
                top2 = rtp.tile([128, ST, E], f32, tag="st2", name="st2")
                nc.vector.tensor_tensor(
                    top2[:], logit[:], m2[:].broadcast_to((128, ST, E)), op=OP.is_ge
                )
                ex2 = rtp.tile([128, ST, E], f32, tag="se2", name="se2")
                nc.vector.tensor_tensor(ex2[:], expl[:], top2[:], op=OP.mult)
                den2 = rtp.tile([128, ST, 1], f32, tag="sd2", name="sd2")
                nc.vector.reduce_sum(den2[:], ex2[:], axis=AX.X)
                rd2 = rtp.tile([128, ST, 1], f32, tag="sr2", name="sr2")
                nc.vector.reciprocal(rd2[:], den2[:])
                gsl = rtp.tile([128, ST, E], f32, tag="sgs", name="sgs")
                nc.vector.tensor_tensor(gsl[:], ex2[:], esel[:, 0:ST, :], op=OP.mult)
                gcol = rtp.tile([128, ST, 1], f32, tag="sgc", name="sgc")
                nc.vector.reduce_sum(gcol[:], gsl[:], axis=AX.X)
                gate_sel = gp.tile([128, ST], f32, tag="gsel", name="gsel")
                gst[b] = gate_sel
                nc.vector.tensor_tensor(
                    gate_sel[:], gcol[:, :, 0], rd2[:, :, 0], op=OP.mult
                )

            def emit_out(b):
                for t in range(ST):
                    ps = psB.tile([128, 512], f32, tag="ps")
                    for dc in range(HC):
                        nc.tensor.matmul(
                            ps[:],
                            ats[b][:, dc, ts(t, 128)],
                            wo_sb[:, dc, :],
                            start=(dc == 0),
                            stop=(dc == HC - 1),
                        )
                    osb = outp.tile([128, H], f32, tag="ot")
                    nc.vector.scalar_tensor_tensor(
                        osb[:],
                        ps[:],
                        rbt[b][:, t : t + 1],
                        fft[b][:, t, :],
                        op0=OP.mult,
                        op1=OP.add,
                    )
                    nc.vector.tensor_scalar_mul(
                        osb[:], osb[:], gst[b][:, t : t + 1]
                    )
                    nc.sync.dma_start(out_d[ts(b * ST + t, 128), :], osb[:])

            def emit_router():
                for g in range(G):
                    xr = xrp.tile([128, HC, 512], f32, tag="xr")
                    for c in range(HC):
                        nc.sync.dma_start(xr[:, c, :], xT32[c][:, ts(g, 512)])
                    lg = psS.tile([128, 4, E], f32, tag="psS")
                    for t in range(4):
                        for c in range(HC):
                            nc.tensor.matmul(
                                lg[:, t, :],
                                xr[:, c, ts(t, 128)],
                                wg_sb[:, c, :],
                                start=(c == 0),
                                stop=(c == HC - 1),
                            )
                    logit = rtp.tile([128, 4, E], f32, tag="lg")
                    nc.vector.tensor_tensor(logit[:], lg[:], bgb[:], op=OP.add)
                    expl = rtp.tile([128, 4, E], f32, tag="ex")
                    nc.scalar.activation(expl[:], logit[:], AF.Exp)
                    sumall = rtp.tile([128, 4, 1], f32, tag="sa")
                    nc.vector.reduce_sum(sumall[:], expl[:], axis=AX.X)
                    rall = rtp.tile([128, 4, 1], f32, tag="ra")
                    nc.vector.reciprocal(rall[:], sumall[:])
                    m1 = rtp.tile([128, 4, 1], f32, tag="m1")
                    nc.vector.reduce_max(m1[:], logit[:], axis=AX.X)
                    mask1 = rtp.tile([128, 4, E], f32, tag="mk")
                    nc.vector.tensor_tensor(
                        mask1[:], logit[:], m1[:].broadcast_to((128, 4, E)), op=OP.is_ge
                    )
                    pmask = rtp.tile([128, 4, E], f32, tag="pm")
                    nc.vector.scalar_tensor_tensor(
                        pmask[:], mask1[:], NEG, logit[:], op0=OP.mult, op1=OP.add
                    )
                    m2 = rtp.tile([128, 4, 1], f32, tag="m2")
                    nc.vector.reduce_max(m2[:], pmask[:], axis=AX.X)
                    top2 = rtp.tile([128, 4, E], f32, tag="t2")
                    nc.vector.tensor_tensor(
                        top2[:], logit[:], m2[:].broadcast_to((128, 4, E)), op=OP.is_ge
                    )
                    psl = rtp.tile([128, 4, E], f32, tag="psl")
                    nc.vector.tensor_tensor(psl[:], expl[:], esel[:], op=OP.mult)
                    pcol = rtp.tile([128, 4, 1], f32, tag="pc")
                    nc.vector.reduce_sum(pcol[:], psl[:], axis=AX.X)
                    pe_t = rtp.tile([128, 4, 1], f32, tag="pe")
                    nc.vector.tensor_tensor(pe_t[:], pcol[:], rall[:], op=OP.mult)
                    nc.vector.tensor_tensor(accp[:], accp[:], pe_t[:, :, 0], op=OP.add)
                    msl = rtp.tile([128, 4, E], f32, tag="msl")
                    nc.vector.tensor_tensor(msl[:], top2[:], esel[:], op=OP.mult)
                    mcol = rtp.tile([128, 4, 1], f32, tag="mc")
                    nc.vector.reduce_sum(mcol[:], msl[:], axis=AX.X)
                    nc.vector.tensor_tensor(accm[:], accm[:], mcol[:, :, 0], op=OP.add)

                nc.sync.dma_start(stats_d[:, 0:4], accm[:])
                nc.sync.dma_start(stats_d[:, 4:8], accp[:])

            # ---------------- emission order ----------------
            emit_kv(0, 0)
            emit_fc1(0)
            emit_kv(0, 1)
            emit_router()
            wq_sb = cp.tile([128, HC, H], bf16, tag="wq")
            for c in range(HC):
                nc.sync.dma_start(wq_sb[:, c, :], wq_d[c])
            w2_sb = cp.tile([128, FC, H], bf16, tag="w2")
            for c in range(FC):
                nc.sync.dma_start(w2_sb[:, c, :], w2_d[c])
            wo_sb = cp.tile([128, HC, H], bf16, tag="wo")
            for c in range(HC):
                nc.sync.dma_start(wo_sb[:, c, :], wo_d[c])
            emit_q(0)
            emit_fc2(0)
            emit_selrouter(0)
            emit_attn(0)
            emit_out(0)
            for b in range(1, B):
                emit_kv(b, 0)
                emit_kv(b, 1)
                emit_fc1(b)
                emit_q(b)
                emit_fc2(b)
                emit_selrouter(b)
                emit_attn(b)
                emit_out(b)

    nc.compile()
    return nc


def _get_nc():
    if "nc" not in _NC_CACHE:
        _NC_CACHE["nc"] = _build_nc()
    return _NC_CACHE["nc"]


def make_in_maps(inputs):
    """Returns (in_maps, idx_list); idx_list is host-side scatter metadata."""
    bf = ml_dtypes.bfloat16
    f32 = np.float32
    x = np.asarray(inputs["x"], f32).reshape(T, H)
    xT = np.ascontiguousarray(x.T)                      # [H, T]
    xT32 = xT.reshape(HC, 128, T)
    xT32_flat = xT
    xT16_flat = np.ascontiguousarray(xT.astype(bf))
    xT16 = xT16_flat.reshape(HC, 128, T)
    Wg = np.asarray(inputs["Wg"], f32)
    bg = np.asarray(inputs["bg"], f32)
    wg = np.ascontiguousarray(Wg).reshape(HC, 128, E)
    bgb = np.ascontiguousarray(
        np.broadcast_to(bg[None, None, :], (128, 4, E)), dtype=f32
    )

    # shadow router (selection only; gates/stats come from the device)
    logits = x @ Wg + bg
    m1 = logits.max(1, keepdims=True)
    pm = np.where(logits >= m1, -np.inf, logits)
    m2 = pm.max(1, keepdims=True)
    top2 = logits >= m2
    expl = np.exp(logits - m1)
    ex2 = expl * top2
    gate_all = ex2 / ex2.sum(1, keepdims=True)          # [T, E]

    in_maps = []
    idx_list = []
    for e in range(E):
        esel = np.zeros((128, 4, E), f32)
        esel[:, :, e] = 1.0
        idx = np.zeros((B, ST * 128), np.int32)
        for b in range(B):
            lo, hi = b * S, (b + 1) * S
            sel = np.nonzero(top2[lo:hi, e])[0] + lo
            if len(sel) > CB:
                keep = np.argsort(gate_all[sel, e])[::-1][:CB]
                sel = np.sort(sel[keep])
            pad_pool = np.nonzero(~top2[lo:hi, e])[0] + lo
            pad = np.full(CB - len(sel), pad_pool[0], np.int32)
            idx[b] = np.concatenate([sel.astype(np.int32), pad])
        idx_list.append(idx.reshape(-1).copy())
        idx_flat = idx.reshape(-1)
        xsel = np.ascontiguousarray(xT16_flat[:, idx_flat]).reshape(HC, 128, B * CB)
        xsel32 = np.ascontiguousarray(xT32_flat[:, idx_flat]).reshape(HC, 128, B * CB)

        w1 = np.ascontiguousarray(np.asarray(inputs["fc1_w"][e], f32).astype(bf)).reshape(HC, 128, F)
        b1 = np.ascontiguousarray(np.asarray(inputs["fc1_b"][e], f32).reshape(FC, 128).T)
        w2 = np.ascontiguousarray(np.asarray(inputs["fc2_w"][e], f32).astype(bf)).reshape(FC, 128, H)
        wq = np.ascontiguousarray(np.asarray(inputs["q_w"][e], f32).astype(bf)).reshape(HC, 128, H)
        wk = np.ascontiguousarray(np.asarray(inputs["k_w"][e], f32).astype(bf)).reshape(HC, 128, H)
        wv = np.ascontiguousarray(np.asarray(inputs["v_w"][e], f32).astype(bf)).reshape(HC, 128, H)
        wo = np.ascontiguousarray(np.asarray(inputs["o_w"][e], f32).astype(bf)).reshape(HC, 128, H)
        qb = np.asarray(inputs["q_b"][e], f32).reshape(HC, 128).T
        kb = np.asarray(inputs["k_b"][e], f32).reshape(HC, 128).T
        qkb = np.ascontiguousarray(np.concatenate([qb, kb], axis=1))
        vbb = np.ascontiguousarray(
            np.broadcast_to(np.asarray(inputs["v_b"][e], f32), (128, H))
        )
        b2ob = np.ascontiguousarray(
            np.broadcast_to(
                np.asarray(inputs["fc2_b"][e], f32) + np.asarray(inputs["o_b"][e], f32),
                (128, H),
            )
        )
        in_maps.append(
            dict(
                xT32=xT32, xT16=xT16, xsel=xsel, xsel32=xsel32,
                wg=wg, bgb=bgb, esel=esel,
                w1=w1, b1=b1, w2=w2, wq=wq, wk=wk, wv=wv, wo=wo,
                qkb=qkb, vbb=vbb, b2ob=b2ob,
            )
        )
    return in_maps, idx_list


def combine(results, idx_list):
    out = np.zeros((T, H), np.float64)
    lb = 0.0
    for e, r in enumerate(results):
        idx_flat = idx_list[e].astype(np.int64)
        np.add.at(out, idx_flat, r["out"].astype(np.float64))
        stats = r["stats"]
        frac = stats[:, 0:4].sum(dtype=np.float64) / T
        imp = stats[:, 4:8].sum(dtype=np.float64) / T
        lb += frac * imp
    lb = np.float32(E * lb)
    return out.astype(np.float32).reshape(B, S, H), lb


def run_spmd(inputs, **kwargs):
    from concourse.bass_utils import run_bass_kernel_spmd

    nc = _get_nc()
    in_maps, idx_list = make_in_maps(inputs)
    res = run_bass_kernel_spmd(nc, in_maps, core_ids=list(range(E)), **kwargs)
    return combine(res.results, idx_list), res


def kernel(**inputs):
    return run_spmd(inputs)[0]


# revision 11
# speedup vs baseline: 1.0163x; 1.0163x over previous
"""Trainium2 Bass kernel: DistributedPiKVMoE (top-2 MoE block, 8 experts).

Expert-parallel with sparse dispatch (8 cores = 8 experts):
  - The host runs a shadow fp32 router only to build per-(expert, batch)
    token index lists (capacity 384 per batch, padded with not-assigned
    tokens) and gathers/transposes the selected tokens' activations.
  - Each core computes, for its expert: the replicated fp32 router
    (exact top-2 gates + load-balance stats), k/v projections over ALL
    tokens (attention keys), and the q/FFN/attention/o pipeline only
    for its selected tokens.  Gates are gathered back from the device
    router output via indirect DMA, so the masked combine uses the
    device's fp32 selection, and padded slots get an exact 0 gate.
  - The host scatter-adds the 8 cores' gated partial outputs (the
    all-reduce of the reference module) and reduces stats -> lb_loss.
  - Expert matmuls run in bf16 with fp32 PSUM accumulation; layouts
    avoid any on-device transpose (x arrives pre-transposed; scores are
    computed transposed [k, q]; softmax via exp + ones-matmul
    denominator; 1/denom and gate fold into the o-projection eviction).

Shapes hardcoded for B=4, S=1024, H=512, E=8, F=2048, TOPK=2.
"""

import numpy as np
import ml_dtypes

B, S, H, E, F = 4, 1024, 512, 8, 2048
T = B * S            # 4096 tokens
HC = H // 128        # 4 h-chunks of 128
FC = F // 128        # 16 f-chunks of 128
G = T // 512         # 8 router token groups of 512
GB = S // 512        # 2 groups per batch
CB = 384             # per-(expert, batch) token capacity (mean ~256)
ST = CB // 128       # 3 subtiles of selected tokens per batch
NEG = -1.0e30

_NC_CACHE = {}


def _build_nc():
    import concourse.tile as tile
    from concourse import bacc, mybir
    from concourse.bass import ts, IndirectOffsetOnAxis

    f32 = mybir.dt.float32
    bf16 = mybir.dt.bfloat16
    i32 = mybir.dt.int32
    AF = mybir.ActivationFunctionType
    OP = mybir.AluOpType
    AX = mybir.AxisListType

    nc = bacc.Bacc("TRN2", target_bir_lowering=False, debug=False)

    # ---- DRAM I/O ----
    xT32 = nc.dram_tensor("xT32", [HC, 128, T], f32, kind="ExternalInput")
    xT16 = nc.dram_tensor("xT16", [HC, 128, T], bf16, kind="ExternalInput")
    xsel_d = nc.dram_tensor("xsel", [HC, 128, B * CB], bf16, kind="ExternalInput")
    xsel32_d = nc.dram_tensor("xsel32", [HC, 128, B * CB], f32, kind="ExternalInput")
    wg_d = nc.dram_tensor("wg", [HC, 128, E], f32, kind="ExternalInput")
    NCST = 32 + 32 + FC + 2 * HC + H + H
    cst_d = nc.dram_tensor("cst", [128, NCST], f32, kind="ExternalInput")
    w1_d = nc.dram_tensor("w1", [HC, 128, F], bf16, kind="ExternalInput")
    w2_d = nc.dram_tensor("w2", [FC, 128, H], bf16, kind="ExternalInput")
    wq_d = nc.dram_tensor("wq", [HC, 128, H], bf16, kind="ExternalInput")
    wk_d = nc.dram_tensor("wk", [HC, 128, H], bf16, kind="ExternalInput")
    wv_d = nc.dram_tensor("wv", [HC, 128, H], bf16, kind="ExternalInput")
    wo_d = nc.dram_tensor("wo", [HC, 128, H], bf16, kind="ExternalInput")
    SCALE = float(np.float32(1.0) / np.sqrt(np.float32(H)))

    out_d = nc.dram_tensor("out", [B * CB, H], f32, kind="ExternalOutput")
    stats_d = nc.dram_tensor("stats", [128, 8], f32, kind="ExternalOutput")
    rscr_d = nc.dram_tensor("rscr", [B, CB], f32, kind="ExternalOutput")

    with tile.TileContext(nc) as tc:
        with (
            tc.tile_pool(name="const", bufs=1) as cp,
            tc.tile_pool(name="xr", bufs=2) as xrp,
            tc.tile_pool(name="xg", bufs=2) as xgp,
            tc.tile_pool(name="xs", bufs=2) as xsp,
            tc.tile_pool(name="h1", bufs=1) as h1p,
            tc.tile_pool(name="ffn", bufs=2) as ffnp,
            tc.tile_pool(name="qq", bufs=2) as qp,
            tc.tile_pool(name="kk", bufs=1) as kp,
            tc.tile_pool(name="vv", bufs=1) as vp,
            tc.tile_pool(name="es", bufs=2) as esp,
            tc.tile_pool(name="at", bufs=2) as atp,
            tc.tile_pool(name="rr", bufs=2) as rp,
            tc.tile_pool(name="gg", bufs=8) as gp,
            tc.tile_pool(name="x3", bufs=2) as x3p,
            tc.tile_pool(name="ou", bufs=4) as outp,
            tc.tile_pool(name="rt", bufs=2) as rtp,
            tc.tile_pool(name="psB", bufs=4, space="PSUM") as psB,
            tc.tile_pool(name="psS", bufs=2, space="PSUM") as psS,
        ):
            # ---- tiny constants: one merged DMA ----
            wg_sb = cp.tile([128, HC, E], f32, tag="wg")
            nc.scalar.dma_start(wg_sb[:], wg_d[:].rearrange("c p e -> p c e"))
            cst = cp.tile([128, NCST], f32, tag="cst")
            nc.scalar.dma_start(cst[:], cst_d[:])
            bgb = cst[:, 0:32].rearrange("p (a e) -> p a e", e=E)
            esel = cst[:, 32:64].rearrange("p (a e) -> p a e", e=E)
            o0 = 64
            b1_sb = cst[:, o0 : o0 + FC]
            qkb = cst[:, o0 + FC : o0 + FC + 2 * HC]
            o1 = o0 + FC + 2 * HC
            vbb = cst[:, o1 : o1 + H]
            b2ob = cst[:, o1 + H : o1 + 2 * H]

            ones_bf = cp.tile([128, 1], bf16, tag="ones")
            nc.vector.memset(ones_bf[:], 1.0)
            accm = cp.tile([128, 4], f32, tag="accm")
            accp = cp.tile([128, 4], f32, tag="accp")
            nc.vector.memset(accm[:], 0.0)
            nc.vector.memset(accp[:], 0.0)

            # ---- big weights (emission position controls DMA priority) ----
            wk_sb = cp.tile([128, HC, H], bf16, tag="wk")
            wv_sb = cp.tile([128, HC, H], bf16, tag="wv")
            nc.scalar.dma_start(wk_sb[:], wk_d[:].rearrange("c p h -> p c h"))
            nc.scalar.dma_start(wv_sb[:], wv_d[:].rearrange("c p h -> p c h"))
            w1_sb = cp.tile([128, HC, F], bf16, tag="w1")
            nc.scalar.dma_start(w1_sb[:], w1_d[:].rearrange("c p f -> p c f"))

            # ---------------- emission helpers ----------------
            kT = {}
            vsb = {}
            xsel = {}
            h1t = {}
            fft = {}
            qTs = {}
            est = {}
            ats = {}
            rbt = {}

            def emit_kv(b, gl):
                g = b * GB + gl
                xg = xgp.tile([128, HC, 512], bf16, tag="xg")
                nc.sync.dma_start(
                    xg[:], xT16[:].rearrange("c p t -> p c t")[:, :, ts(g, 512)]
                )
                if gl == 0:
                    kT[b] = kp.tile([128, HC, S], bf16, tag="kT", name="kTt")
                    vsb[b] = vp.tile([128, 8, H], bf16, tag="v", name="vt")
                for dc in range(HC):
                    ps = psB.tile([128, 512], f32, tag="ps")
                    for c in range(HC):
                        nc.tensor.matmul(
                            ps[:],
                            wk_sb[:, c, ts(dc, 128)],
                            xg[:, c, :],
                            start=(c == 0),
                            stop=(c == HC - 1),
                        )
                    nc.vector.tensor_scalar_add(
                        kT[b][:, dc, ts(gl, 512)], ps[:], qkb[:, HC + dc : HC + dc + 1]
                    )
                for t in range(4):
                    ps = psB.tile([128, 512], f32, tag="ps")
                    for c in range(HC):
                        nc.tensor.matmul(
                            ps[:],
                            xg[:, c, ts(t, 128)],
                            wv_sb[:, c, :],
                            start=(c == 0),
                            stop=(c == HC - 1),
                        )
                    nc.vector.tensor_tensor(
                        vsb[b][:, gl * 4 + t, :], ps[:], vbb[:], op=OP.add
                    )

            def emit_fc1(b):
                xs = xsp.tile([128, HC, CB], bf16, tag="xs")
                xsel[b] = xs
                nc.sync.dma_start(
                    xs[:], xsel_d[:].rearrange("c p t -> p c t")[:, :, ts(b, CB)]
                )
                h1 = h1p.tile([128, FC, CB], bf16, tag="h1")
                h1t[b] = h1
                for fc in range(FC):
                    ps = psB.tile([128, 512], f32, tag="ps")
                    for c in range(HC):
                        nc.tensor.matmul(
                            ps[:, :CB],
                            w1_sb[:, c, ts(fc, 128)],
                            xs[:, c, :],
                            start=(c == 0),
                            stop=(c == HC - 1),
                        )
                    nc.scalar.activation(
                        h1[:, fc, :], ps[:, :CB], AF.Gelu, bias=b1_sb[:, fc : fc + 1]
                    )

            def emit_fc2(b):
                h1 = h1t[b]
                fsb = ffnp.tile([128, ST, H], f32, tag="ffn")
                fft[b] = fsb
                for t in range(ST):
                    ps = psB.tile([128, 512], f32, tag="ps")
                    for fc in range(FC):
                        nc.tensor.matmul(
                            ps[:],
                            h1[:, fc, ts(t, 128)],
                            w2_sb[:, fc, :],
                            start=(fc == 0),
                            stop=(fc == FC - 1),
                        )
                    nc.vector.tensor_tensor(fsb[:, t, :], ps[:], b2ob[:], op=OP.add)

            def emit_q(b):
                xs = xsel[b]
                qT = qp.tile([128, HC, CB], bf16, tag="qT")
                qTs[b] = qT
                for dc in range(HC):
                    ps = psB.tile([128, 512], f32, tag="ps")
                    for c in range(HC):
                        nc.tensor.matmul(
                            ps[:, :CB],
                            wq_sb[:, c, ts(dc, 128)],
                            xs[:, c, :],
                            start=(c == 0),
                            stop=(c == HC - 1),
                        )
                    nc.vector.tensor_scalar(
                        qT[:, dc, :],
                        ps[:, :CB],
                        qkb[:, dc : dc + 1],
                        SCALE,
                        op0=OP.add,
                        op1=OP.mult,
                    )

            def emit_attn(b):
                es = esp.tile([128, 8, CB], bf16, tag="es")
                est[b] = es
                for kc in range(8):
                    ps = psB.tile([128, 512], f32, tag="ps")
                    for dc in range(HC):
                        nc.tensor.matmul(
                            ps[:, :CB],
                            kT[b][:, dc, ts(kc, 128)],
                            qTs[b][:, dc, :],
                            start=(dc == 0),
                            stop=(dc == HC - 1),
                        )
                    nc.scalar.activation(es[:, kc, :], ps[:, :CB], AF.Exp)
                # softmax denominators for the selected queries
                rrow = rp.tile([1, CB], f32, tag="rrow")
                psd = psS.tile([1, 512], f32, tag="psS")
                for kc in range(8):
                    nc.tensor.matmul(
                        psd[:, :CB],
                        ones_bf[:],
                        es[:, kc, :],
                        start=(kc == 0),
                        stop=(kc == 7),
                    )
                nc.vector.reciprocal(rrow[:], psd[:, :CB])
                nc.sync.dma_start(rscr_d[b : b + 1, :], rrow[:])
                rb = rp.tile([128, ST], f32, tag="rb")
                rbt[b] = rb
                nc.sync.dma_start(
                    rb[:], rscr_d[b : b + 1, :].rearrange("a (c p) -> (a p) c", p=128)
                )
                at = atp.tile([128, HC, CB], bf16, tag="at")
                ats[b] = at
                for dc in range(HC):
                    ps = psB.tile([128, 512], f32, tag="ps")
                    for kc in range(8):
                        nc.tensor.matmul(
                            ps[:, :CB],
                            vsb[b][:, kc, ts(dc, 128)],
                            es[:, kc, :],
                            start=(kc == 0),
                            stop=(kc == 7),
                        )
                    nc.vector.tensor_copy(at[:, dc, :], ps[:, :CB])

            gst = {}

            def emit_selrouter(b):
                xs32 = x3p.tile([128, HC, CB], f32, tag="xs32", name="xs32t")
                nc.sync.dma_start(
                    xs32[:], xsel32_d[:].rearrange("c p t -> p c t")[:, :, ts(b, CB)]
                )
                lg = psS.tile([128, ST, E], f32, tag="psS", name="lgs")
                for t in range(ST):
                    for c in range(HC):
                        nc.tensor.matmul(
                            lg[:, t, :],
                            xs32[:, c, ts(t, 128)],
                            wg_sb[:, c, :],
                            start=(c == 0),
                            stop=(c == HC - 1),
                        )
                logit = rtp.tile([128, ST, E], f32, tag="slg", name="slg")
                nc.vector.tensor_tensor(logit[:], lg[:], bgb[:, 0:ST, :], op=OP.add)
                expl = rtp.tile([128, ST, E], f32, tag="sex", name="sex")
                nc.scalar.activation(expl[:], logit[:], AF.Exp)
                m1 = rtp.tile([128, ST, 1], f32, tag="sm1", name="sm1")
                nc.vector.reduce_max(m1[:], logit[:], axis=AX.X)
                mask1 = rtp.tile([128, ST, E], f32, tag="smk", name="smk")
                nc.vector.tensor_tensor(
                    mask1[:], logit[:], m1[:].broadcast_to((128, ST, E)), op=OP.is_ge
                )
                pmask = rtp.tile([128, ST, E], f32, tag="spm", name="spm")
                nc.vector.scalar_tensor_tensor(
                    pmask[:], mask1[:], NEG, logit[:], op0=OP.mult, op1=OP.add
                )
                m2 = rtp.tile([128, ST, 1], f32, tag="sm2", name="sm2")
                nc.vector.reduce_max(m2[:], pmask[:], axis=AX.X)
                top2 = rtp.tile([128, ST, E], f32, tag="st2", name="st2")
                nc.vector.tensor_tensor(
                    top2[:], logit[:], m2[:].broadcast_to((128, ST, E)), op=OP.is_ge
                )
                ex2 = rtp.tile([128, ST, E], f32, tag="se2", name="se2")
                nc.vector.tensor_tensor(ex2[:], expl[:], top2[:], op=OP.mult)
                den2 = rtp.tile([128, ST, 1], f32, tag="sd2", name="sd2")
                nc.vector.reduce_sum(den2[:], ex2[:], axis=AX.X)
                rd2 = rtp.tile([128, ST, 1], f32, tag="sr2", name="sr2")
                nc.vector.reciprocal(rd2[:], den2[:])
                gsl = rtp.tile([128, ST, E], f32, tag="sgs", name="sgs")
                nc.vector.tensor_tensor(gsl[:], ex2[:], esel[:, 0:ST, :], op=OP.mult)
                gcol = rtp.tile([128, ST, 1], f32, tag="sgc", name="sgc")
                nc.vector.reduce_sum(gcol[:], gsl[:], axis=AX.X)
                gate_sel = gp.tile([128, ST], f32, tag="gsel", name="gsel")
                gst[b] = gate_sel
                nc.vector.tensor_tensor(
                    gate_sel[:], gcol[:, :, 0], rd2[:, :, 0], op=OP.mult
                )

            def emit_out(b):
                for t in range(ST):
                    ps = psB.tile([128, 512], f32, tag="ps")
                    for dc in range(HC):
                        nc.tensor.matmul(
                            ps[:],
                            ats[b][:, dc, ts(t, 128)],
                            wo_sb[:, dc, :],
                            start=(dc == 0),
                            stop=(dc == HC - 1),
                        )
                    osb = outp.tile([128, H], f32, tag="ot")
                    nc.vector.scalar_tensor_tensor(
                        osb[:],
                        ps[:],
                        rbt[b][:, t : t + 1],
                        fft[b][:, t, :],
                        op0=OP.mult,
                        op1=OP.add,
                    )
                    nc.vector.tensor_scalar_mul(
                        osb[:], osb[:], gst[b][:, t : t + 1]
                    )
                    nc.sync.dma_start(out_d[ts(b * ST + t, 128), :], osb[:])

            def emit_router():
                for g in range(G):
                    xr = xrp.tile([128, HC, 512], f32, tag="xr")
                    nc.sync.dma_start(
                        xr[:], xT32[:].rearrange("c p t -> p c t")[:, :, ts(g, 512)]
                    )
                    lg = psS.tile([128, 4, E], f32, tag="psS")
                    for t in range(4):
                        for c in range(HC):
                            nc.tensor.matmul(
                                lg[:, t, :],
                                xr[:, c, ts(t, 128)],
                                wg_sb[:, c, :],
                                start=(c == 0),
                                stop=(c == HC - 1),
                            )
                    logit = rtp.tile([128, 4, E], f32, tag="lg")
                    nc.vector.tensor_tensor(logit[:], lg[:], bgb[:], op=OP.add)
                    expl = rtp.tile([128, 4, E], f32, tag="ex")
                    nc.scalar.activation(expl[:], logit[:], AF.Exp)
                    sumall = rtp.tile([128, 4, 1], f32, tag="sa")
                    nc.vector.reduce_sum(sumall[:], expl[:], axis=AX.X)
                    rall = rtp.tile([128, 4, 1], f32, tag="ra")
                    nc.vector.reciprocal(rall[:], sumall[:])
                    m1 = rtp.tile([128, 4, 1], f32, tag="m1")
                    nc.vector.reduce_max(m1[:], logit[:], axis=AX.X)
                    mask1 = rtp.tile([128, 4, E], f32, tag="mk")
                    nc.vector.tensor_tensor(
                        mask1[:], logit[:], m1[:].broadcast_to((128, 4, E)), op=OP.is_ge
                    )
                    pmask = rtp.tile([128, 4, E], f32, tag="pm")
                    nc.vector.scalar_tensor_tensor(
                        pmask[:], mask1[:], NEG, logit[:], op0=OP.mult, op1=OP.add
                    )
                    m2 = rtp.tile([128, 4, 1], f32, tag="m2")
                    nc.vector.reduce_max(m2[:], pmask[:], axis=AX.X)
                    top2 = rtp.tile([128, 4, E], f32, tag="t2")
                    nc.vector.tensor_tensor(
                        top2[:], logit[:], m2[:].broadcast_to((128, 4, E)), op=OP.is_ge
                    )
                    psl = rtp.tile([128, 4, E], f32, tag="psl")
                    nc.vector.tensor_tensor(psl[:], expl[:], esel[:], op=OP.mult)
                    pcol = rtp.tile([128, 4, 1], f32, tag="pc")
                    nc.vector.reduce_sum(pcol[:], psl[:], axis=AX.X)
                    pe_t = rtp.tile([128, 4, 1], f32, tag="pe")
                    nc.vector.tensor_tensor(pe_t[:], pcol[:], rall[:], op=OP.mult)
                    nc.vector.tensor_tensor(accp[:], accp[:], pe_t[:, :, 0], op=OP.add)
                    msl = rtp.tile([128, 4, E], f32, tag="msl")
                    nc.vector.tensor_tensor(msl[:], top2[:], esel[:], op=OP.mult)
                    mcol = rtp.tile([128, 4, 1], f32, tag="mc")
                    nc.vector.reduce_sum(mcol[:], msl[:], axis=AX.X)
                    nc.vector.tensor_tensor(accm[:], accm[:], mcol[:, :, 0], op=OP.add)

                nc.sync.dma_start(stats_d[:, 0:4], accm[:])
                nc.sync.dma_start(stats_d[:, 4:8], accp[:])

            # ---------------- emission order ----------------
            emit_kv(0, 0)
            emit_fc1(0)
            emit_kv(0, 1)
            emit_router()
            wq_sb = cp.tile([128, HC, H], bf16, tag="wq")
            nc.scalar.dma_start(wq_sb[:], wq_d[:].rearrange("c p h -> p c h"))
            w2_sb = cp.tile([128, FC, H], bf16, tag="w2")
            nc.scalar.dma_start(w2_sb[:], w2_d[:].rearrange("c p f -> p c f"))
            wo_sb = cp.tile([128, HC, H], bf16, tag="wo")
            nc.scalar.dma_start(wo_sb[:], wo_d[:].rearrange("c p h -> p c h"))
            emit_q(0)
            emit_fc2(0)
            emit_selrouter(0)
            emit_attn(0)
            emit_out(0)
            for b in range(1, B):
                emit_kv(b, 0)
                emit_kv(b, 1)
                emit_fc1(b)
                emit_q(b)
                emit_fc2(b)
                emit_selrouter(b)
                emit_attn(b)
                emit_out(b)

    nc.compile()
    return nc


def _get_nc():
    if "nc" not in _NC_CACHE:
        _NC_CACHE["nc"] = _build_nc()
    return _NC_CACHE["nc"]


def make_in_maps(inputs):
    """Returns (in_maps, idx_list); idx_list is host-side scatter metadata."""
    bf = ml_dtypes.bfloat16
    f32 = np.float32
    x = np.asarray(inputs["x"], f32).reshape(T, H)
    xT = np.ascontiguousarray(x.T)                      # [H, T]
    xT32 = xT.reshape(HC, 128, T)
    xT32_flat = xT
    xT16_flat = np.ascontiguousarray(xT.astype(bf))
    xT16 = xT16_flat.reshape(HC, 128, T)
    Wg = np.asarray(inputs["Wg"], f32)
    bg = np.asarray(inputs["bg"], f32)
    wg = np.ascontiguousarray(Wg).reshape(HC, 128, E)
    bgb = np.ascontiguousarray(
        np.broadcast_to(bg[None, None, :], (128, 4, E)), dtype=f32
    )

    # shadow router (selection only; gates/stats come from the device)
    logits = x @ Wg + bg
    m1 = logits.max(1, keepdims=True)
    pm = np.where(logits >= m1, -np.inf, logits)
    m2 = pm.max(1, keepdims=True)
    top2 = logits >= m2
    expl = np.exp(logits - m1)
    ex2 = expl * top2
    gate_all = ex2 / ex2.sum(1, keepdims=True)          # [T, E]

    in_maps = []
    idx_list = []
    for e in range(E):
        esel = np.zeros((128, 4, E), f32)
        esel[:, :, e] = 1.0
        idx = np.zeros((B, ST * 128), np.int32)
        for b in range(B):
            lo, hi = b * S, (b + 1) * S
            sel = np.nonzero(top2[lo:hi, e])[0] + lo
            if len(sel) > CB:
                keep = np.argsort(gate_all[sel, e])[::-1][:CB]
                sel = np.sort(sel[keep])
            pad_pool = np.nonzero(~top2[lo:hi, e])[0] + lo
            pad = np.full(CB - len(sel), pad_pool[0], np.int32)
            idx[b] = np.concatenate([sel.astype(np.int32), pad])
        idx_list.append(idx.reshape(-1).copy())
        idx_flat = idx.reshape(-1)
        xsel = np.ascontiguousarray(xT16_flat[:, idx_flat]).reshape(HC, 128, B * CB)
        xsel32 = np.ascontiguousarray(xT32_flat[:, idx_flat]).reshape(HC, 128, B * CB)

        w1 = np.ascontiguousarray(np.asarray(inputs["fc1_w"][e], f32).astype(bf)).reshape(HC, 128, F)
        b1 = np.ascontiguousarray(np.asarray(inputs["fc1_b"][e], f32).reshape(FC, 128).T)
        w2 = np.ascontiguousarray(np.asarray(inputs["fc2_w"][e], f32).astype(bf)).reshape(FC, 128, H)
        wq = np.ascontiguousarray(np.asarray(inputs["q_w"][e], f32).astype(bf)).reshape(HC, 128, H)
        wk = np.ascontiguousarray(np.asarray(inputs["k_w"][e], f32).astype(bf)).reshape(HC, 128, H)
        wv = np.ascontiguousarray(np.asarray(inputs["v_w"][e], f32).astype(bf)).reshape(HC, 128, H)
        wo = np.ascontiguousarray(np.asarray(inputs["o_w"][e], f32).astype(bf)).reshape(HC, 128, H)
        qb = np.asarray(inputs["q_b"][e], f32).reshape(HC, 128).T
        kb = np.asarray(inputs["k_b"][e], f32).reshape(HC, 128).T
        qkb = np.concatenate([qb, kb], axis=1)
        vbb = np.broadcast_to(np.asarray(inputs["v_b"][e], f32), (128, H))
        b2ob = np.broadcast_to(
            np.asarray(inputs["fc2_b"][e], f32) + np.asarray(inputs["o_b"][e], f32),
            (128, H),
        )
        cst = np.ascontiguousarray(
            np.concatenate(
                [bgb.reshape(128, 32), esel.reshape(128, 32), b1, qkb, vbb, b2ob],
                axis=1,
            ),
            dtype=f32,
        )
        in_maps.append(
            dict(
                xT32=xT32, xT16=xT16, xsel=xsel, xsel32=xsel32,
                wg=wg, cst=cst,
                w1=w1, w2=w2, wq=wq, wk=wk, wv=wv, wo=wo,
            )
        )
    return in_maps, idx_list


def combine(results, idx_list):
    out = np.zeros((T, H), np.float64)
    lb = 0.0
    for e, r in enumerate(results):
        idx_flat = idx_list[e].astype(np.int64)
        np.add.at(out, idx_flat, r["out"].astype(np.float64))
        stats = r["stats"]
        frac = stats[:, 0:4].sum(dtype=np.float64) / T
        imp = stats[:, 4:8].sum(dtype=np.float64) / T
        lb += frac * imp
    lb = np.float32(E * lb)
    return out.astype(np.float32).reshape(B, S, H), lb


def run_spmd(inputs, **kwargs):
    from concourse.bass_utils import run_bass_kernel_spmd

    nc = _get_nc()
    in_maps, idx_list = make_in_maps(inputs)
    res = run_bass_kernel_spmd(nc, in_maps, core_ids=list(range(E)), **kwargs)
    return combine(res.results, idx_list), res


def kernel(**inputs):
    return run_spmd(inputs)[0]
